# revision 24
# baseline (speedup 1.0000x reference)
"""GTE contrastive loss kernel for 8 Trainium2 NeuronCores — v3.

Math (reference): loss = -mean_i( cos(a_i,p_i)/T - log(partition_i) ),
partition_i = sum_j E_ap[i,j] + sum_j E_aa[i,j] + sum_j E_ap[j,i]
            + sum_j E_pp[j,i] - (self_a)_i - (self_p)_i,
E_xy = exp(cos/T).  The self terms are exp(20*||x_hat_fp16||^2) computed
from the SAME fp16-rounded vectors the matmuls consume, so the device's
huge diagonal terms cancel exactly on the host at any matmul precision.

Sharding: core k owns row block k (1024 rows); inputs host-rotated by
-1024k rows (one SPMD program).  Column block j = global block (k+j)%8.
aa/pp symmetry: blocks 0..4; blocks 1..3 also emit column sums that
become the missing row-sum pieces on other cores.

v3 engine plan (driven by the v2 HW trace):
 - ALL transposes on the DMA engines: a 2-byte XBAR DMA-transpose of a
   [128, 128] fp16 slab (two adjacent 128-row tiles) yields the two
   transposed tiles STACKED in the partition dim (tile 2q on partitions
   0:64, tile 2q+1 on 64:128).  xTs keeps that stacked layout.
 - matmuls are fp16, K=128, with zero-padded stationaries (built by
   small partition-moving SBUF->SBUF DMAs): an "even" matmul carries the
   m-tile dims on K rows 0:64 (zeros below) so only the even tiles of
   the moving slab contribute; the "odd" one mirrors it.  A fused block
   pair (j, j+1) needs just 2 matmuls + 2 ldweights per [128, 2048]
   PSUM tile with 1024-wide moving operands.  Columns inside each block
   come out parity-permuted; the host unpermutes the column sums (row
   sums are order-invariant).
 - 9 block-pairs x 8 row-tiles, one [128,2048] Exp + f32 accum_out each.
 - exp output bf16; per-block column sums accumulate on the DVE (2-byte
   2x mode, strided slices); partition reduce on GPSIMD (axis=C).
 - startup pipelined in 16-tile chunks woven between the first pairs;
   a-tiles 40..64 are never used (aa stops at block 4) and are skipped.
"""

import os
import sys

import numpy as np

for _p in ("/opt/trn_rl_repo", os.path.expanduser("/root/.axon_site/_ro/trn_rl_repo")):
    if os.path.isdir(_p) and _p not in sys.path:
        sys.path.insert(0, _p)

from concourse import bass, tile  # noqa: E402
from concourse.bass_utils import run_bass_kernel_spmd  # noqa: E402

mybir = bass.mybir
F32 = mybir.dt.float32
F16 = mybir.dt.float16
BF16 = mybir.dt.bfloat16

N, D, NCORES = 8192, 64, 8
B = N // NCORES            # 1024 rows per core
MT = B // 128              # 8 row tiles of 128
TFULL = N // 128           # 64 tiles
INV_T = 20.0

AP_BLOCKS = list(range(8))
SYM_BLOCKS = [0, 1, 2, 3, 4]
SYM_COL_BLOCKS = [1, 2, 3]

PAIRS = [
    [("ap", 0), ("ap", 1)],
    [("ap", 2), ("ap", 3)],
    [("ap", 4), ("ap", 5)],
    [("ap", 6), ("ap", 7)],
    [("aa", 0), ("aa", 1)],
    [("aa", 2), ("aa", 3)],
    [("pp", 0), ("pp", 1)],
    [("pp", 2), ("pp", 3)],
    [("aa", 4), ("pp", 4)],
]
NSLOT = len(PAIRS)

_COLACC = {}
for _j in AP_BLOCKS:
    _COLACC[("ap", _j)] = _j
for _i, _j in enumerate(SYM_COL_BLOCKS):
    _COLACC[("aa", _j)] = 8 + _i
    _COLACC[("pp", _j)] = 11 + _i
NCS = len(_COLACC)

AT_T = 40                  # a tiles actually used (aT cols < 5120)
CH = 16                    # pipeline chunk, in 128-row tiles

# Column permutation inside one 1024-col block: raw slot s (as stored in
# colacc / cs outputs) holds block-local column PERM_1024[s].
# s = parity*512 + tau*128 + r  ->  col (2*tau + parity)*128 + r
PERM_1024 = np.empty(1024, np.int64)
for _s in range(1024):
    _par, _tau, _r = _s // 512, (_s % 512) // 128, _s % 128
    PERM_1024[_s] = (2 * _tau + _par) * 128 + _r


def ST_IDX(si, m):
    return si * MT + m


class _Input:
    """Chunked DMA -> normalize(fp16 out) -> stacked DMA-transpose."""

    def __init__(self, nc, ldp, xts, sqr, dram_in, name, tmax, sq_eng, mul_eng):
        self.nc, self.name, self.tmax = nc, name, tmax
        self.sq_eng, self.mul_eng = sq_eng, mul_eng
        self.sqr = sqr
        self.nat = ldp.tile([128, tmax, D], F32, tag=f"{name}_nat")
        self.nat2h = ldp.tile([128, tmax, D], F16, tag=f"{name}_nat2h")
        self.ss = ldp.tile([128, tmax], F32, tag=f"{name}_ss")
        self.nrm = ldp.tile([128, tmax], F32, tag=f"{name}_nrm")
        self.inv = ldp.tile([128, tmax], F32, tag=f"{name}_inv")
        self.src = dram_in[:].rearrange("(t p) d -> p t d", p=128)
        # stacked transposed layout: col q*128+r holds, on partitions
        # 0:64, dims of tile 2q row r; on 64:128, dims of tile 2q+1.
        self.xTs = xts.tile([128, (tmax // 2) * 128], F16, tag=f"{name}_xTs")

    def dma(self, c0, c1):
        self.nc.sync.dma_start(out=self.nat[:, c0:c1, :],
                               in_=self.src[:, c0:c1, :])

    def chunk(self, c0, c1, tr_eng):
        nc = self.nc
        w = c1 - c0
        sq = self.sqr.tile([128, CH, D], F32, tag="sq_ring")
        self.sq_eng.tensor_mul(sq[:, 0:w, :], self.nat[:, c0:c1, :],
                               self.nat[:, c0:c1, :])
        nc.vector.tensor_reduce(self.ss[:, c0:c1], sq[:, 0:w, :],
                                axis=mybir.AxisListType.X,
                                op=mybir.AluOpType.add)
        # rsqrt entirely on the DVE (an ACT Sqrt here head-of-line blocks
        # the exp stream in ACT's strict FIFO): bit-trick seed + 2 Newton
        # steps gives ~1e-6 relative — far below the fp16 rounding the
        # ssq self-term correction already absorbs.
        ssv = self.ss[:, c0:c1]
        yv = self.inv[:, c0:c1]
        yi = yv.bitcast(mybir.dt.int32)
        # seed: 0x5f3759df - (bits(x) >> 1)  ==  (bits(x)>>1)*-1 + C
        nc.vector.tensor_scalar(yi, ssv.bitcast(mybir.dt.int32),
                                1, None, mybir.AluOpType.arith_shift_right)
        nc.vector.tensor_scalar(yi, yi, -1, 0x5f3759df,
                                mybir.AluOpType.mult, mybir.AluOpType.add)
        t0 = self.sqr.tile([128, CH], F32, tag="nrs_ring")
        t0v = t0[:, 0:w]
        for _ in range(2):
            nc.vector.tensor_mul(t0v, yv, yv)            # y^2
            nc.vector.tensor_mul(t0v, t0v, ssv)          # x*y^2
            nc.vector.tensor_scalar(t0v, t0v, -0.5, 1.5,
                                    mybir.AluOpType.mult,
                                    mybir.AluOpType.add)  # 1.5-0.5xy^2
            nc.vector.tensor_mul(yv, yv, t0v)            # y *= corr
        inv_b = yv.broadcast_to([128, w, D])
        self.mul_eng.tensor_mul(self.nat2h[:, c0:c1, :],
                                self.nat[:, c0:c1, :], inv_b)
        for q0 in range(c0 // 2, c1 // 2):
            tr_eng.dma_start(
                out=self.xTs[:, q0 * 128:(q0 + 1) * 128],
                in_=self.nat2h[:, 2 * q0:2 * q0 + 2, :],
                transpose=True,
            )


def build_program():
    nc = bass.Bass()
    a_in = nc.declare_dram_parameter("a", [N, D], F32, isOutput=False)
    p_in = nc.declare_dram_parameter("p", [N, D], F32, isOutput=False)
    o_st = nc.declare_dram_parameter("st", [128, MT * NSLOT], F32, isOutput=True)
    o_diag = nc.declare_dram_parameter("diag", [128, MT], F32, isOutput=True)
    o_ssa = nc.declare_dram_parameter("ssq_a", [128, MT], F32, isOutput=True)
    o_ssp = nc.declare_dram_parameter("ssq_p", [128, MT], F32, isOutput=True)
    o_cs_ap = nc.declare_dram_parameter("cs_ap", [1, 8 * B], F32, isOutput=True)
    o_cs_aa = nc.declare_dram_parameter("cs_aa", [1, 3 * B], F32, isOutput=True)
    o_cs_pp = nc.declare_dram_parameter("cs_pp", [1, 3 * B], F32, isOutput=True)
    dsts = {"ap": o_cs_ap, "aa": o_cs_aa, "pp": o_cs_pp}

    with tile.TileContext(nc) as tc:
        import contextlib

        with contextlib.ExitStack() as ctx:
            res = ctx.enter_context(tc.tile_pool(name="results", bufs=1))
            st = res.tile([128, MT * NSLOT], F32)
            diag = res.tile([128, MT], F32)
            colacc = res.tile([128, NCS, B], BF16)
            ones_bf = res.tile([128, 128], BF16)
            nc.vector.memset(ones_bf[:], 1.0)

            xts = ctx.enter_context(tc.tile_pool(name="xts", bufs=1))
            ldp = ctx.enter_context(tc.tile_pool(name="ld", bufs=1))
            sqr = ctx.enter_context(tc.tile_pool(name="sqr", bufs=2))
            csp = ctx.enter_context(tc.tile_pool(name="csstage", bufs=2))
            mmp = ctx.enter_context(tc.tile_pool(name="mm", bufs=2, space="PSUM"))
            ep = ctx.enter_context(tc.tile_pool(name="etile", bufs=3))

            A = _Input(nc, ldp, xts, sqr, a_in, "a", AT_T,
                       nc.vector, nc.vector)
            P = _Input(nc, ldp, xts, sqr, p_in, "p", TFULL,
                       nc.gpsimd, nc.gpsimd)

            A.dma(0, CH)
            P.dma(0, CH)
            P.dma(CH, 2 * CH)
            P.dma(2 * CH, 3 * CH)
            P.dma(3 * CH, 4 * CH)
            A.dma(CH, 2 * CH)
            A.dma(2 * CH, AT_T)

            # Zero-padded K=128 stationaries: stat[par][:, m*128:(m+1)*128]
            # has the m-tile dims on K rows 64*par .. 64*par+64, 0 elsewhere.
            # Built with partition-moving SBUF->SBUF DMAs from xTs.
            stats = {}
            for inp, nm in ((A, "a"), (P, "p")):
                se = res.tile([128, MT * 128], F16, tag=f"stat_{nm}_e")
                so = res.tile([128, MT * 128], F16, tag=f"stat_{nm}_o")
                nc.vector.memset(se[:], 0.0)
                nc.vector.memset(so[:], 0.0)
                stats[nm] = (se, so)

            def emit_stats(inp, nm, eng):
                se, so = stats[nm]
                for m in range(MT):
                    src = inp.xTs[64 * (m % 2):64 * (m % 2) + 64,
                                  (m // 2) * 128:(m // 2 + 1) * 128]
                    eng.dma_start(
                        out=se[0:64, m * 128:(m + 1) * 128], in_=src)
                    eng.dma_start(
                        out=so[64:128, m * 128:(m + 1) * 128], in_=src)

            def emit_pair(si, pair):
                (matL, jL), (matR, jR) = pair
                fused = (matL == matR and jR == jL + 1)
                xi, yi = {"ap": (A, P), "aa": (A, A), "pp": (P, P)}[matL]
                se, so = stats["a" if xi is A else "p"]
                for m in range(MT):
                    mm_ps = mmp.tile([128, 2 * B], F32, tag="mm")
                    if fused:
                        # K=64 row-tiled: T0 (partitions 0:64, even tiles)
                        # and T8 (64:128, odd tiles) co-stream in the
                        # 64x128-tiled PE array.
                        for c in range(2):
                            for par, stt in ((0, se), (1, so)):
                                h0 = 64 * par
                                nc.tensor.matmul(
                                    mm_ps[:, par * B + c * 512:
                                          par * B + (c + 1) * 512],
                                    stt[h0:h0 + 64, m * 128:(m + 1) * 128],
                                    yi.xTs[h0:h0 + 64,
                                           (jL + c) * 512:(jL + c + 1) * 512],
                                    start=True, stop=True,
                                    tile_position=(h0, 0))
                    else:
                        # pair8: two independent 512-wide blocks
                        for h, (mat, j) in enumerate(pair):
                            xh, yh = {"ap": (A, P), "aa": (A, A),
                                      "pp": (P, P)}[mat]
                            seh, soh = stats["a" if xh is A else "p"]
                            for par, stt in ((0, seh), (1, soh)):
                                h0 = 64 * par
                                o0 = h * B + par * 512
                                nc.tensor.matmul(
                                    mm_ps[:, o0:o0 + 512],
                                    stt[h0:h0 + 64, m * 128:(m + 1) * 128],
                                    yh.xTs[h0:h0 + 64, j * 512:(j + 1) * 512],
                                    start=True, stop=True,
                                    tile_position=(h0, 0))
                    e = ep.tile([128, 2 * B], BF16, tag="e")
                    slot = ST_IDX(si, m)
                    nc.scalar.activation(
                        e[:], mm_ps[:], mybir.ActivationFunctionType.Exp,
                        scale=INV_T,
                        accum_out=st[:, slot:slot + 1],
                    )
                    if fused:
                        for h, (mat, j) in enumerate(pair):
                            ci = _COLACC.get((mat, j))
                            if ci is None:
                                continue
                            # block h: even half at h*512, odd at B+h*512
                            # (two contiguous [128,512] ops; strided 3D
                            # APs measured 30-100x slower on DVE)
                            for par in range(2):
                                eh = e[:, par * B + h * 512:
                                       par * B + (h + 1) * 512]
                                ca = colacc[:, ci, par * 512:(par + 1) * 512]
                                if m == 0:
                                    nc.vector.tensor_copy(ca, eh)
                                else:
                                    nc.vector.tensor_add(ca, ca, eh)
                # column-sum partition reduce: ones-matmul in the mm ring
                halves = [(h, mat, j, _COLACC.get((mat, j)))
                          for h, (mat, j) in enumerate(pair)]
                halves = [x for x in halves if x[3] is not None]
                if not halves:
                    return
                cred = mmp.tile([128, 2 * B], F32, tag="mm")
                for i, (h, mat, j, ci) in enumerate(halves):
                    for c in range(2):
                        nc.tensor.matmul(
                            cred[:, i * B + c * 512:i * B + (c + 1) * 512],
                            ones_bf[:],
                            colacc[:, ci, c * 512:(c + 1) * 512],
                            start=True, stop=True,
                        )
                for i, (h, mat, j, ci) in enumerate(halves):
                    cstage = csp.tile([1, B], F32, tag="cs")
                    nc.vector.tensor_copy(cstage[:], cred[0:1, i * B:(i + 1) * B])
                    cj = j if mat == "ap" else SYM_COL_BLOCKS.index(j)
                    nc.sync.dma_start(out=dsts[mat][0:1, cj * B:(cj + 1) * B],
                                      in_=cstage[:])

            # ---- woven schedule ----
            A.chunk(0, CH, nc.scalar)
            P.chunk(0, CH, nc.sync)
            emit_stats(A, "a", nc.scalar)
            emit_stats(P, "p", nc.sync)

            # diag cos(a_i,p_i) from raw f32 tiles + inverse norms
            dtmp = sqr.tile([128, CH, D], F32, tag="sq_ring")
            nc.vector.tensor_mul(dtmp[:, 0:MT, :], A.nat[:, 0:MT, :],
                                 P.nat[:, 0:MT, :])
            dots = sqr.tile([128, MT], F32, tag="ssq_ring")
            nc.vector.tensor_reduce(dots[:], dtmp[:, 0:MT, :],
                                    axis=mybir.AxisListType.X,
                                    op=mybir.AluOpType.add)
            nc.vector.tensor_mul(dots[:], dots[:], A.inv[:, 0:MT])
            nc.vector.tensor_mul(diag[:], dots[:], P.inv[:, 0:MT])

            # machine-matched self terms from the fp16 normalized tiles
            for inp, o_ssq in ((A, o_ssa), (P, o_ssp)):
                sqh = sqr.tile([128, MT, D], F32, tag="sq_ring")
                nc.vector.tensor_mul(sqh[:], inp.nat2h[:, 0:MT, :],
                                     inp.nat2h[:, 0:MT, :])
                ssq = sqr.tile([128, MT], F32, tag="ssq_ring")
                nc.vector.tensor_reduce(ssq[:], sqh[:],
                                        axis=mybir.AxisListType.X,
                                        op=mybir.AluOpType.add)
                nc.sync.dma_start(out=o_ssq[:], in_=ssq[:])

            # chunks are emitted two pairs before first use so their
            # DMA/Pool/DVE chains never stall the ACT exp stream
            P.chunk(CH, 2 * CH, nc.sync)
            emit_pair(0, PAIRS[0])
            P.chunk(2 * CH, 3 * CH, nc.sync)
            emit_pair(1, PAIRS[1])
            P.chunk(3 * CH, 4 * CH, nc.sync)
            emit_pair(2, PAIRS[2])
            A.chunk(CH, 2 * CH, nc.sync)
            emit_pair(3, PAIRS[3])
            emit_pair(4, PAIRS[4])
            A.chunk(2 * CH, AT_T, nc.sync)
            emit_pair(5, PAIRS[5])
            emit_pair(6, PAIRS[6])
            emit_pair(7, PAIRS[7])
            emit_pair(8, PAIRS[8])

            nc.sync.dma_start(out=o_st[:], in_=st[:])
            nc.sync.dma_start(out=o_diag[:], in_=diag[:])
    return nc


def combine(core_outs):
    """core_outs: list (per core) of dicts with the 9 output arrays."""
    rs = np.empty(N, np.float64)
    diag = np.empty(N, np.float32)
    self_terms = np.empty(N, np.float64)
    cs_ap_tot = np.zeros(N, np.float64)
    aa_contrib = np.zeros(N, np.float64)
    pp_contrib = np.zeros(N, np.float64)

    def unperm(vec_b):
        out = np.empty(B, np.float64)
        out[PERM_1024] = vec_b
        return out

    for k, o in enumerate(core_outs):
        sl = slice(k * B, (k + 1) * B)
        rs[sl] = o["st"].reshape(128, NSLOT, MT).astype(np.float64).sum(1) \
                        .T.reshape(B)
        diag[sl] = o["diag"].T.reshape(B)
        self_terms[sl] = (
            np.exp(INV_T * o["ssq_a"].astype(np.float64)) +
            np.exp(INV_T * o["ssq_p"].astype(np.float64))
        ).T.reshape(B)

        cs_ap = np.concatenate(
            [unperm(o["cs_ap"].reshape(8, B)[j]) for j in range(8)])
        cs_ap_tot += np.roll(cs_ap, k * B)
        for row, j in enumerate(SYM_COL_BLOCKS):
            v = np.zeros(N, np.float64)
            v[j * B:(j + 1) * B] = unperm(o["cs_aa"].reshape(3, B)[row])
            aa_contrib += np.roll(v, k * B)
            v = np.zeros(N, np.float64)
            v[j * B:(j + 1) * B] = unperm(o["cs_pp"].reshape(3, B)[row])
            pp_contrib += np.roll(v, k * B)

    partition = (rs + cs_ap_tot + aa_contrib + pp_contrib - self_terms)
    pos_logit = INV_T * diag.astype(np.float64)
    loss = -(pos_logit - np.log(partition)).mean()
    return np.float32(loss)


def _split_waits(nc):
    """Walrus codegen allows ~1 sync wait per instruction; hoist extra
    waits onto same-engine NoOps inserted just before the instruction."""
    for fn in nc.m.functions:
        for blk in fn.blocks:
            new = []
            for inst in blk.instructions:
                si = getattr(inst, "sync_info", None)
                keep = 1
                if si is not None and si.on_wait and len(si.on_wait) > keep:
                    waits = list(si.on_wait)
                    for i, w in enumerate(waits[:-keep]):
                        nop = mybir.InstNoOp(name=f"{inst.name}-sw{i}")
                        nop.engine = inst.engine
                        nop.sync_info = mybir.SyncInfo(on_wait=[w], on_update=[])
                        new.append(nop)
                    inst.sync_info = mybir.SyncInfo(
                        on_wait=list(waits[-keep:]),
                        on_update=list(si.on_update))
                new.append(inst)
            blk.instructions = new


_NC_CACHE = None


def _get_program():
    global _NC_CACHE
    if _NC_CACHE is None:
        _NC_CACHE = build_program()
        _split_waits(_NC_CACHE)
    return _NC_CACHE


def run(anchor_embeddings, positive_embeddings, trace=False, **trace_kwargs):
    a = np.ascontiguousarray(anchor_embeddings, dtype=np.float32)
    p = np.ascontiguousarray(positive_embeddings, dtype=np.float32)
    in_maps = [
        {"a": np.roll(a, -k * B, axis=0), "p": np.roll(p, -k * B, axis=0)}
        for k in range(NCORES)
    ]
    nc = _get_program()
    res = run_bass_kernel_spmd(nc, in_maps, list(range(NCORES)), trace=trace,
                               **trace_kwargs)
    return combine(res.results), res


def kernel(anchor_embeddings, positive_embeddings):
    loss, _ = run(anchor_embeddings, positive_embeddings)
    return loss


# revision 26
# speedup vs baseline: 1.0742x; 1.0742x over previous
"""GTE contrastive loss kernel for 8 Trainium2 NeuronCores — v3.

Math (reference): loss = -mean_i( cos(a_i,p_i)/T - log(partition_i) ),
partition_i = sum_j E_ap[i,j] + sum_j E_aa[i,j] + sum_j E_ap[j,i]
            + sum_j E_pp[j,i] - (self_a)_i - (self_p)_i,
E_xy = exp(cos/T).  The self terms are exp(20*||x_hat_fp16||^2) computed
from the SAME fp16-rounded vectors the matmuls consume, so the device's
huge diagonal terms cancel exactly on the host at any matmul precision.

Sharding: core k owns row block k (1024 rows); inputs host-rotated by
-1024k rows (one SPMD program).  Column block j = global block (k+j)%8.
aa/pp symmetry: blocks 0..4; blocks 1..3 also emit column sums that
become the missing row-sum pieces on other cores.

v3 engine plan (driven by the v2 HW trace):
 - ALL transposes on the DMA engines: a 2-byte XBAR DMA-transpose of a
   [128, 128] fp16 slab (two adjacent 128-row tiles) yields the two
   transposed tiles STACKED in the partition dim (tile 2q on partitions
   0:64, tile 2q+1 on 64:128).  xTs keeps that stacked layout.
 - matmuls are fp16, K=128, with zero-padded stationaries (built by
   small partition-moving SBUF->SBUF DMAs): an "even" matmul carries the
   m-tile dims on K rows 0:64 (zeros below) so only the even tiles of
   the moving slab contribute; the "odd" one mirrors it.  A fused block
   pair (j, j+1) needs just 2 matmuls + 2 ldweights per [128, 2048]
   PSUM tile with 1024-wide moving operands.  Columns inside each block
   come out parity-permuted; the host unpermutes the column sums (row
   sums are order-invariant).
 - 9 block-pairs x 8 row-tiles, one [128,2048] Exp + f32 accum_out each.
 - exp output bf16; per-block column sums accumulate on the DVE (2-byte
   2x mode, strided slices); partition reduce on GPSIMD (axis=C).
 - startup pipelined in 16-tile chunks woven between the first pairs;
   a-tiles 40..64 are never used (aa stops at block 4) and are skipped.
"""

import os
import sys

import numpy as np

for _p in ("/opt/trn_rl_repo", os.path.expanduser("/root/.axon_site/_ro/trn_rl_repo")):
    if os.path.isdir(_p) and _p not in sys.path:
        sys.path.insert(0, _p)

from concourse import bass, tile  # noqa: E402
from concourse.bass_utils import run_bass_kernel_spmd  # noqa: E402

mybir = bass.mybir
F32 = mybir.dt.float32
F16 = mybir.dt.float16
BF16 = mybir.dt.bfloat16

N, D, NCORES = 8192, 64, 8
B = N // NCORES            # 1024 rows per core
MT = B // 128              # 8 row tiles of 128
TFULL = N // 128           # 64 tiles
INV_T = 20.0

AP_BLOCKS = list(range(8))
SYM_BLOCKS = [0, 1, 2, 3, 4]
SYM_COL_BLOCKS = [1, 2, 3]

PAIRS = [
    [("ap", 0), ("ap", 1)],
    [("ap", 2), ("ap", 3)],
    [("ap", 4), ("ap", 5)],
    [("ap", 6), ("ap", 7)],
    [("aa", 0), ("aa", 1)],
    [("aa", 2), ("aa", 3)],
    [("pp", 0), ("pp", 1)],
    [("pp", 2), ("pp", 3)],
    [("aa", 4), ("pp", 4)],
]
NSLOT = len(PAIRS)

_COLACC = {}
for _j in AP_BLOCKS:
    _COLACC[("ap", _j)] = _j
for _i, _j in enumerate(SYM_COL_BLOCKS):
    _COLACC[("aa", _j)] = 8 + _i
    _COLACC[("pp", _j)] = 11 + _i
NCS = len(_COLACC)

AT_T = 40                  # a tiles actually used (aT cols < 5120)
CH = 16                    # pipeline chunk, in 128-row tiles

# Column permutation inside one 1024-col block: raw slot s (as stored in
# colacc / cs outputs) holds block-local column PERM_1024[s].
# s = parity*512 + tau*128 + r  ->  col (2*tau + parity)*128 + r
PERM_1024 = np.empty(1024, np.int64)
for _s in range(1024):
    _par, _tau, _r = _s // 512, (_s % 512) // 128, _s % 128
    PERM_1024[_s] = (2 * _tau + _par) * 128 + _r


def ST_IDX(si, m):
    return si * MT + m


class _Input:
    """Chunked DMA -> normalize(fp16 out) -> stacked DMA-transpose."""

    def __init__(self, nc, ldp, xts, sqr, dram_in, name, tmax, sq_eng, mul_eng):
        self.nc, self.name, self.tmax = nc, name, tmax
        self.sq_eng, self.mul_eng = sq_eng, mul_eng
        self.sqr = sqr
        self.nat = ldp.tile([128, tmax, D], F32, tag=f"{name}_nat")
        self.nat2h = ldp.tile([128, tmax, D], F16, tag=f"{name}_nat2h")
        self.ss = ldp.tile([128, tmax], F32, tag=f"{name}_ss")
        self.nrm = ldp.tile([128, tmax], F32, tag=f"{name}_nrm")
        self.inv = ldp.tile([128, tmax], F32, tag=f"{name}_inv")
        self.src = dram_in[:].rearrange("(t p) d -> p t d", p=128)
        # stacked transposed layout: col q*128+r holds, on partitions
        # 0:64, dims of tile 2q row r; on 64:128, dims of tile 2q+1.
        self.xTs = xts.tile([128, (tmax // 2) * 128], F16, tag=f"{name}_xTs")

    def dma(self, c0, c1):
        self.nc.sync.dma_start(out=self.nat[:, c0:c1, :],
                               in_=self.src[:, c0:c1, :])

    def chunk(self, c0, c1, tr_eng):
        nc = self.nc
        w = c1 - c0
        sq = self.sqr.tile([128, CH, D], F32, tag="sq_ring")
        self.sq_eng.tensor_mul(sq[:, 0:w, :], self.nat[:, c0:c1, :],
                               self.nat[:, c0:c1, :])
        nc.vector.tensor_reduce(self.ss[:, c0:c1], sq[:, 0:w, :],
                                axis=mybir.AxisListType.X,
                                op=mybir.AluOpType.add)
        # rsqrt entirely on the DVE (an ACT Sqrt here head-of-line blocks
        # the exp stream in ACT's strict FIFO): bit-trick seed + 2 Newton
        # steps gives ~1e-6 relative — far below the fp16 rounding the
        # ssq self-term correction already absorbs.
        ssv = self.ss[:, c0:c1]
        yv = self.inv[:, c0:c1]
        yi = yv.bitcast(mybir.dt.int32)
        # seed: 0x5f3759df - (bits(x) >> 1)  ==  (bits(x)>>1)*-1 + C
        nc.vector.tensor_scalar(yi, ssv.bitcast(mybir.dt.int32),
                                1, None, mybir.AluOpType.arith_shift_right)
        nc.vector.tensor_scalar(yi, yi, -1, 0x5f3759df,
                                mybir.AluOpType.mult, mybir.AluOpType.add)
        t0 = self.sqr.tile([128, CH], F32, tag="nrs_ring")
        t0v = t0[:, 0:w]
        for _ in range(2):
            nc.vector.tensor_mul(t0v, yv, yv)            # y^2
            nc.vector.tensor_mul(t0v, t0v, ssv)          # x*y^2
            nc.vector.tensor_scalar(t0v, t0v, -0.5, 1.5,
                                    mybir.AluOpType.mult,
                                    mybir.AluOpType.add)  # 1.5-0.5xy^2
            nc.vector.tensor_mul(yv, yv, t0v)            # y *= corr
        inv_b = yv.broadcast_to([128, w, D])
        self.mul_eng.tensor_mul(self.nat2h[:, c0:c1, :],
                                self.nat[:, c0:c1, :], inv_b)
        for q0 in range(c0 // 2, c1 // 2):
            tr_eng.dma_start(
                out=self.xTs[:, q0 * 128:(q0 + 1) * 128],
                in_=self.nat2h[:, 2 * q0:2 * q0 + 2, :],
                transpose=True,
            )


def build_program():
    nc = bass.Bass()
    a_in = nc.declare_dram_parameter("a", [N, D], F32, isOutput=False)
    p_in = nc.declare_dram_parameter("p", [N, D], F32, isOutput=False)
    o_st = nc.declare_dram_parameter("st", [128, MT * NSLOT], F32, isOutput=True)
    o_diag = nc.declare_dram_parameter("diag", [128, MT], F32, isOutput=True)
    o_ssa = nc.declare_dram_parameter("ssq_a", [128, MT], F32, isOutput=True)
    o_ssp = nc.declare_dram_parameter("ssq_p", [128, MT], F32, isOutput=True)
    o_cs_ap = nc.declare_dram_parameter("cs_ap", [1, 8 * B], F32, isOutput=True)
    o_cs_aa = nc.declare_dram_parameter("cs_aa", [1, 3 * B], F32, isOutput=True)
    o_cs_pp = nc.declare_dram_parameter("cs_pp", [1, 3 * B], F32, isOutput=True)
    dsts = {"ap": o_cs_ap, "aa": o_cs_aa, "pp": o_cs_pp}

    with tile.TileContext(nc) as tc:
        import contextlib

        with contextlib.ExitStack() as ctx:
            res = ctx.enter_context(tc.tile_pool(name="results", bufs=1))
            st = res.tile([128, MT * NSLOT], F32)
            diag = res.tile([128, MT], F32)
            colacc = res.tile([128, NCS, B], BF16)
            ones_bf = res.tile([128, 128], BF16)
            nc.vector.memset(ones_bf[:], 1.0)

            xts = ctx.enter_context(tc.tile_pool(name="xts", bufs=1))
            ldp = ctx.enter_context(tc.tile_pool(name="ld", bufs=1))
            sqr = ctx.enter_context(tc.tile_pool(name="sqr", bufs=2))
            csp = ctx.enter_context(tc.tile_pool(name="csstage", bufs=4))
            mmp = ctx.enter_context(tc.tile_pool(name="mm", bufs=2, space="PSUM"))
            ep = ctx.enter_context(tc.tile_pool(name="etile", bufs=3))

            A = _Input(nc, ldp, xts, sqr, a_in, "a", AT_T,
                       nc.vector, nc.vector)
            P = _Input(nc, ldp, xts, sqr, p_in, "p", TFULL,
                       nc.gpsimd, nc.gpsimd)

            A.dma(0, CH)
            P.dma(0, CH)
            P.dma(CH, 2 * CH)
            P.dma(2 * CH, 3 * CH)
            P.dma(3 * CH, 4 * CH)
            A.dma(CH, 2 * CH)
            A.dma(2 * CH, AT_T)

            # Zero-padded K=128 stationaries: stat[par][:, m*128:(m+1)*128]
            # has the m-tile dims on K rows 64*par .. 64*par+64, 0 elsewhere.
            # Built with partition-moving SBUF->SBUF DMAs from xTs.
            stats = {}
            for inp, nm in ((A, "a"), (P, "p")):
                se = res.tile([128, MT * 128], F16, tag=f"stat_{nm}_e")
                so = res.tile([128, MT * 128], F16, tag=f"stat_{nm}_o")
                nc.vector.memset(se[:], 0.0)
                nc.vector.memset(so[:], 0.0)
                stats[nm] = (se, so)

            def emit_stats(inp, nm, eng):
                se, so = stats[nm]
                for m in range(MT):
                    src = inp.xTs[64 * (m % 2):64 * (m % 2) + 64,
                                  (m // 2) * 128:(m // 2 + 1) * 128]
                    eng.dma_start(
                        out=se[0:64, m * 128:(m + 1) * 128], in_=src)
                    eng.dma_start(
                        out=so[64:128, m * 128:(m + 1) * 128], in_=src)

            def emit_pair(si, pair):
                (matL, jL), (matR, jR) = pair
                fused = (matL == matR and jR == jL + 1)
                xi, yi = {"ap": (A, P), "aa": (A, A), "pp": (P, P)}[matL]
                se, so = stats["a" if xi is A else "p"]
                for m in range(MT):
                    mm_ps = mmp.tile([128, 2 * B], F32, tag="mm")
                    if fused:
                        # K=64 row-tiled: T0 (partitions 0:64, even tiles)
                        # and T8 (64:128, odd tiles) co-stream in the
                        # 64x128-tiled PE array.
                        for c in range(2):
                            for par, stt in ((0, se), (1, so)):
                                h0 = 64 * par
                                nc.tensor.matmul(
                                    mm_ps[:, par * B + c * 512:
                                          par * B + (c + 1) * 512],
                                    stt[h0:h0 + 64, m * 128:(m + 1) * 128],
                                    yi.xTs[h0:h0 + 64,
                                           (jL + c) * 512:(jL + c + 1) * 512],
                                    start=True, stop=True,
                                    tile_position=(h0, 0))
                    else:
                        # pair8: two independent 512-wide blocks
                        for h, (mat, j) in enumerate(pair):
                            xh, yh = {"ap": (A, P), "aa": (A, A),
                                      "pp": (P, P)}[mat]
                            seh, soh = stats["a" if xh is A else "p"]
                            for par, stt in ((0, seh), (1, soh)):
                                h0 = 64 * par
                                o0 = h * B + par * 512
                                nc.tensor.matmul(
                                    mm_ps[:, o0:o0 + 512],
                                    stt[h0:h0 + 64, m * 128:(m + 1) * 128],
                                    yh.xTs[h0:h0 + 64, j * 512:(j + 1) * 512],
                                    start=True, stop=True,
                                    tile_position=(h0, 0))
                    e = ep.tile([128, 2 * B], BF16, tag="e")
                    slot = ST_IDX(si, m)
                    nc.scalar.activation(
                        e[:], mm_ps[:], mybir.ActivationFunctionType.Exp,
                        scale=INV_T,
                        accum_out=st[:, slot:slot + 1],
                    )
                    if fused:
                        for h, (mat, j) in enumerate(pair):
                            ci = _COLACC.get((mat, j))
                            if ci is None:
                                continue
                            # block h: even half at h*512, odd at B+h*512
                            # (two contiguous [128,512] ops; strided 3D
                            # APs measured 30-100x slower on DVE)
                            for par in range(2):
                                eh = e[:, par * B + h * 512:
                                       par * B + (h + 1) * 512]
                                ca = colacc[:, ci, par * 512:(par + 1) * 512]
                                if m == 0:
                                    nc.vector.tensor_copy(ca, eh)
                                else:
                                    nc.vector.tensor_add(ca, ca, eh)
                # column-sum partition reduce: ones-matmul in the mm ring
                halves = [(h, mat, j, _COLACC.get((mat, j)))
                          for h, (mat, j) in enumerate(pair)]
                halves = [x for x in halves if x[3] is not None]
                if not halves:
                    return
                cred = mmp.tile([128, 2 * B], F32, tag="mm")
                for i, (h, mat, j, ci) in enumerate(halves):
                    for c in range(2):
                        nc.tensor.matmul(
                            cred[:, i * B + c * 512:i * B + (c + 1) * 512],
                            ones_bf[:],
                            colacc[:, ci, c * 512:(c + 1) * 512],
                            start=True, stop=True,
                        )
                for i, (h, mat, j, ci) in enumerate(halves):
                    cstage = csp.tile([1, B], F32, tag="cs")
                    nc.vector.tensor_copy(cstage[:], cred[0:1, i * B:(i + 1) * B])
                    cj = j if mat == "ap" else SYM_COL_BLOCKS.index(j)
                    # gpsimd software-DGE queue: the SP queue carries the
                    # bulk chunk transposes, which would head-of-line
                    # block this small DMA and stall the cstage ring
                    nc.gpsimd.dma_start(out=dsts[mat][0:1, cj * B:(cj + 1) * B],
                                        in_=cstage[:])

            # ---- woven schedule ----
            A.chunk(0, CH, nc.scalar)
            P.chunk(0, CH, nc.sync)
            emit_stats(A, "a", nc.scalar)
            emit_stats(P, "p", nc.sync)

            # diag cos(a_i,p_i) from raw f32 tiles + inverse norms
            dtmp = sqr.tile([128, CH, D], F32, tag="sq_ring")
            nc.vector.tensor_mul(dtmp[:, 0:MT, :], A.nat[:, 0:MT, :],
                                 P.nat[:, 0:MT, :])
            dots = sqr.tile([128, MT], F32, tag="ssq_ring")
            nc.vector.tensor_reduce(dots[:], dtmp[:, 0:MT, :],
                                    axis=mybir.AxisListType.X,
                                    op=mybir.AluOpType.add)
            nc.vector.tensor_mul(dots[:], dots[:], A.inv[:, 0:MT])
            nc.vector.tensor_mul(diag[:], dots[:], P.inv[:, 0:MT])

            # machine-matched self terms from the fp16 normalized tiles
            for inp, o_ssq in ((A, o_ssa), (P, o_ssp)):
                sqh = sqr.tile([128, MT, D], F32, tag="sq_ring")
                nc.vector.tensor_mul(sqh[:], inp.nat2h[:, 0:MT, :],
                                     inp.nat2h[:, 0:MT, :])
                ssq = sqr.tile([128, MT], F32, tag="ssq_ring")
                nc.vector.tensor_reduce(ssq[:], sqh[:],
                                        axis=mybir.AxisListType.X,
                                        op=mybir.AluOpType.add)
                nc.sync.dma_start(out=o_ssq[:], in_=ssq[:])

            # remaining chunks all emitted after pair0: their DVE/Pool
            # chain work lands early in those queues and the bulky SP
            # transposes spread across pairs 1-3, long before use
            P.chunk(CH, 2 * CH, nc.sync)
            emit_pair(0, PAIRS[0])
            P.chunk(2 * CH, 3 * CH, nc.sync)
            P.chunk(3 * CH, 4 * CH, nc.sync)
            A.chunk(CH, 2 * CH, nc.sync)
            A.chunk(2 * CH, AT_T, nc.sync)
            emit_pair(1, PAIRS[1])
            emit_pair(2, PAIRS[2])
            emit_pair(3, PAIRS[3])
            emit_pair(4, PAIRS[4])
            emit_pair(5, PAIRS[5])
            emit_pair(6, PAIRS[6])
            emit_pair(7, PAIRS[7])
            emit_pair(8, PAIRS[8])

            nc.sync.dma_start(out=o_st[:], in_=st[:])
            nc.sync.dma_start(out=o_diag[:], in_=diag[:])
    return nc


def combine(core_outs):
    """core_outs: list (per core) of dicts with the 9 output arrays."""
    rs = np.empty(N, np.float64)
    diag = np.empty(N, np.float32)
    self_terms = np.empty(N, np.float64)
    cs_ap_tot = np.zeros(N, np.float64)
    aa_contrib = np.zeros(N, np.float64)
    pp_contrib = np.zeros(N, np.float64)

    def unperm(vec_b):
        out = np.empty(B, np.float64)
        out[PERM_1024] = vec_b
        return out

    for k, o in enumerate(core_outs):
        sl = slice(k * B, (k + 1) * B)
        rs[sl] = o["st"].reshape(128, NSLOT, MT).astype(np.float64).sum(1) \
                        .T.reshape(B)
        diag[sl] = o["diag"].T.reshape(B)
        self_terms[sl] = (
            np.exp(INV_T * o["ssq_a"].astype(np.float64)) +
            np.exp(INV_T * o["ssq_p"].astype(np.float64))
        ).T.reshape(B)

        cs_ap = np.concatenate(
            [unperm(o["cs_ap"].reshape(8, B)[j]) for j in range(8)])
        cs_ap_tot += np.roll(cs_ap, k * B)
        for row, j in enumerate(SYM_COL_BLOCKS):
            v = np.zeros(N, np.float64)
            v[j * B:(j + 1) * B] = unperm(o["cs_aa"].reshape(3, B)[row])
            aa_contrib += np.roll(v, k * B)
            v = np.zeros(N, np.float64)
            v[j * B:(j + 1) * B] = unperm(o["cs_pp"].reshape(3, B)[row])
            pp_contrib += np.roll(v, k * B)

    partition = (rs + cs_ap_tot + aa_contrib + pp_contrib - self_terms)
    pos_logit = INV_T * diag.astype(np.float64)
    loss = -(pos_logit - np.log(partition)).mean()
    return np.float32(loss)


def _split_waits(nc):
    """Walrus codegen allows ~1 sync wait per instruction; hoist extra
    waits onto same-engine NoOps inserted just before the instruction."""
    for fn in nc.m.functions:
        for blk in fn.blocks:
            new = []
            for inst in blk.instructions:
                si = getattr(inst, "sync_info", None)
                keep = 1
                if si is not None and si.on_wait and len(si.on_wait) > keep:
                    waits = list(si.on_wait)
                    for i, w in enumerate(waits[:-keep]):
                        nop = mybir.InstNoOp(name=f"{inst.name}-sw{i}")
                        nop.engine = inst.engine
                        nop.sync_info = mybir.SyncInfo(on_wait=[w], on_update=[])
                        new.append(nop)
                    inst.sync_info = mybir.SyncInfo(
                        on_wait=list(waits[-keep:]),
                        on_update=list(si.on_update))
                new.append(inst)
            blk.instructions = new


_NC_CACHE = None


def _get_program():
    global _NC_CACHE
    if _NC_CACHE is None:
        _NC_CACHE = build_program()
        _split_waits(_NC_CACHE)
    return _NC_CACHE


def run(anchor_embeddings, positive_embeddings, trace=False, **trace_kwargs):
    a = np.ascontiguousarray(anchor_embeddings, dtype=np.float32)
    p = np.ascontiguousarray(positive_embeddings, dtype=np.float32)
    in_maps = [
        {"a": np.roll(a, -k * B, axis=0), "p": np.roll(p, -k * B, axis=0)}
        for k in range(NCORES)
    ]
    nc = _get_program()
    res = run_bass_kernel_spmd(nc, in_maps, list(range(NCORES)), trace=trace,
                               **trace_kwargs)
    return combine(res.results), res


def kernel(anchor_embeddings, positive_embeddings):
    loss, _ = run(anchor_embeddings, positive_embeddings)
    return loss


# revision 27
# speedup vs baseline: 1.2109x; 1.1273x over previous
"""GTE contrastive loss kernel for 8 Trainium2 NeuronCores — v3.

Math (reference): loss = -mean_i( cos(a_i,p_i)/T - log(partition_i) ),
partition_i = sum_j E_ap[i,j] + sum_j E_aa[i,j] + sum_j E_ap[j,i]
            + sum_j E_pp[j,i] - (self_a)_i - (self_p)_i,
E_xy = exp(cos/T).  The self terms are exp(20*||x_hat_fp16||^2) computed
from the SAME fp16-rounded vectors the matmuls consume, so the device's
huge diagonal terms cancel exactly on the host at any matmul precision.

Sharding: core k owns row block k (1024 rows); inputs host-rotated by
-1024k rows (one SPMD program).  Column block j = global block (k+j)%8.
aa/pp symmetry: blocks 0..4; blocks 1..3 also emit column sums that
become the missing row-sum pieces on other cores.

v3 engine plan (driven by the v2 HW trace):
 - ALL transposes on the DMA engines: a 2-byte XBAR DMA-transpose of a
   [128, 128] fp16 slab (two adjacent 128-row tiles) yields the two
   transposed tiles STACKED in the partition dim (tile 2q on partitions
   0:64, tile 2q+1 on 64:128).  xTs keeps that stacked layout.
 - matmuls are fp16, K=128, with zero-padded stationaries (built by
   small partition-moving SBUF->SBUF DMAs): an "even" matmul carries the
   m-tile dims on K rows 0:64 (zeros below) so only the even tiles of
   the moving slab contribute; the "odd" one mirrors it.  A fused block
   pair (j, j+1) needs just 2 matmuls + 2 ldweights per [128, 2048]
   PSUM tile with 1024-wide moving operands.  Columns inside each block
   come out parity-permuted; the host unpermutes the column sums (row
   sums are order-invariant).
 - 9 block-pairs x 8 row-tiles, one [128,2048] Exp + f32 accum_out each.
 - exp output bf16; per-block column sums accumulate on the DVE (2-byte
   2x mode, strided slices); partition reduce on GPSIMD (axis=C).
 - startup pipelined in 16-tile chunks woven between the first pairs;
   a-tiles 40..64 are never used (aa stops at block 4) and are skipped.
"""

import os
import sys

import numpy as np

for _p in ("/opt/trn_rl_repo", os.path.expanduser("/root/.axon_site/_ro/trn_rl_repo")):
    if os.path.isdir(_p) and _p not in sys.path:
        sys.path.insert(0, _p)

from concourse import bass, tile  # noqa: E402
from concourse.bass_utils import run_bass_kernel_spmd  # noqa: E402

mybir = bass.mybir
F32 = mybir.dt.float32
F16 = mybir.dt.float16
BF16 = mybir.dt.bfloat16

N, D, NCORES = 8192, 64, 8
B = N // NCORES            # 1024 rows per core
MT = B // 128              # 8 row tiles of 128
TFULL = N // 128           # 64 tiles
INV_T = 20.0

AP_BLOCKS = list(range(8))
SYM_BLOCKS = [0, 1, 2, 3, 4]
SYM_COL_BLOCKS = [1, 2, 3]

PAIRS = [
    [("ap", 0), ("ap", 1)],
    [("ap", 2), ("ap", 3)],
    [("ap", 4), ("ap", 5)],
    [("ap", 6), ("ap", 7)],
    [("aa", 0), ("aa", 1)],
    [("aa", 2), ("aa", 3)],
    [("pp", 0), ("pp", 1)],
    [("pp", 2), ("pp", 3)],
    [("aa", 4), ("pp", 4)],
]
NSLOT = len(PAIRS)

_COLACC = {}
for _j in AP_BLOCKS:
    _COLACC[("ap", _j)] = _j
for _i, _j in enumerate(SYM_COL_BLOCKS):
    _COLACC[("aa", _j)] = 8 + _i
    _COLACC[("pp", _j)] = 11 + _i
NCS = len(_COLACC)

AT_T = 40                  # a tiles actually used (aT cols < 5120)
CH = 16                    # pipeline chunk, in 128-row tiles

# Column permutation inside one 1024-col block: raw slot s (as stored in
# colacc / cs outputs) holds block-local column PERM_1024[s].
# s = parity*512 + tau*128 + r  ->  col (2*tau + parity)*128 + r
PERM_1024 = np.empty(1024, np.int64)
for _s in range(1024):
    _par, _tau, _r = _s // 512, (_s % 512) // 128, _s % 128
    PERM_1024[_s] = (2 * _tau + _par) * 128 + _r


def ST_IDX(si, m):
    return si * MT + m


class _Input:
    """Chunked DMA -> normalize(fp16 out) -> stacked DMA-transpose."""

    def __init__(self, nc, ldp, xts, sqr, dram_in, name, tmax, sq_eng, mul_eng):
        self.nc, self.name, self.tmax = nc, name, tmax
        self.sq_eng, self.mul_eng = sq_eng, mul_eng
        self.sqr = sqr
        self.nat = ldp.tile([128, tmax, D], F32, tag=f"{name}_nat")
        self.nat2h = ldp.tile([128, tmax, D], F16, tag=f"{name}_nat2h")
        self.ss = ldp.tile([128, tmax], F32, tag=f"{name}_ss")
        self.nrm = ldp.tile([128, tmax], F32, tag=f"{name}_nrm")
        self.inv = ldp.tile([128, tmax], F32, tag=f"{name}_inv")
        self.src = dram_in[:].rearrange("(t p) d -> p t d", p=128)
        # stacked transposed layout: col q*128+r holds, on partitions
        # 0:64, dims of tile 2q row r; on 64:128, dims of tile 2q+1.
        self.xTs = xts.tile([128, (tmax // 2) * 128], F16, tag=f"{name}_xTs")

    def dma(self, c0, c1):
        self.nc.sync.dma_start(out=self.nat[:, c0:c1, :],
                               in_=self.src[:, c0:c1, :])

    def chunk(self, c0, c1, tr_eng):
        nc = self.nc
        w = c1 - c0
        sq = self.sqr.tile([128, CH, D], F32, tag="sq_ring")
        self.sq_eng.tensor_mul(sq[:, 0:w, :], self.nat[:, c0:c1, :],
                               self.nat[:, c0:c1, :])
        nc.vector.tensor_reduce(self.ss[:, c0:c1], sq[:, 0:w, :],
                                axis=mybir.AxisListType.X,
                                op=mybir.AluOpType.add)
        # rsqrt entirely on the DVE (an ACT Sqrt here head-of-line blocks
        # the exp stream in ACT's strict FIFO): bit-trick seed + 2 Newton
        # steps gives ~1e-6 relative — far below the fp16 rounding the
        # ssq self-term correction already absorbs.
        ssv = self.ss[:, c0:c1]
        yv = self.inv[:, c0:c1]
        yi = yv.bitcast(mybir.dt.int32)
        # seed: 0x5f3759df - (bits(x) >> 1)  ==  (bits(x)>>1)*-1 + C
        nc.vector.tensor_scalar(yi, ssv.bitcast(mybir.dt.int32),
                                1, None, mybir.AluOpType.arith_shift_right)
        nc.vector.tensor_scalar(yi, yi, -1, 0x5f3759df,
                                mybir.AluOpType.mult, mybir.AluOpType.add)
        t0 = self.sqr.tile([128, CH], F32, tag="nrs_ring")
        t0v = t0[:, 0:w]
        for _ in range(2):
            nc.vector.tensor_mul(t0v, yv, yv)            # y^2
            nc.vector.tensor_mul(t0v, t0v, ssv)          # x*y^2
            nc.vector.tensor_scalar(t0v, t0v, -0.5, 1.5,
                                    mybir.AluOpType.mult,
                                    mybir.AluOpType.add)  # 1.5-0.5xy^2
            nc.vector.tensor_mul(yv, yv, t0v)            # y *= corr
        inv_b = yv.broadcast_to([128, w, D])
        self.mul_eng.tensor_mul(self.nat2h[:, c0:c1, :],
                                self.nat[:, c0:c1, :], inv_b)
        for q0 in range(c0 // 2, c1 // 2):
            tr_eng.dma_start(
                out=self.xTs[:, q0 * 128:(q0 + 1) * 128],
                in_=self.nat2h[:, 2 * q0:2 * q0 + 2, :],
                transpose=True,
            )


def build_program():
    nc = bass.Bass()
    a_in = nc.declare_dram_parameter("a", [N, D], F32, isOutput=False)
    p_in = nc.declare_dram_parameter("p", [N, D], F32, isOutput=False)
    o_st = nc.declare_dram_parameter("st", [128, MT * NSLOT], F32, isOutput=True)
    o_diag = nc.declare_dram_parameter("diag", [128, MT], F32, isOutput=True)
    o_ssa = nc.declare_dram_parameter("ssq_a", [128, MT], F32, isOutput=True)
    o_ssp = nc.declare_dram_parameter("ssq_p", [128, MT], F32, isOutput=True)
    o_cacc = nc.declare_dram_parameter("colacc", [128, NCS * B], BF16,
                                       isOutput=True)

    with tile.TileContext(nc) as tc:
        import contextlib

        with contextlib.ExitStack() as ctx:
            res = ctx.enter_context(tc.tile_pool(name="results", bufs=1))
            st = res.tile([128, MT * NSLOT], F32)
            diag = res.tile([128, MT], F32)
            colacc = res.tile([128, NCS, B], BF16)
            ones_bf = res.tile([128, 128], BF16)
            nc.vector.memset(ones_bf[:], 1.0)

            xts = ctx.enter_context(tc.tile_pool(name="xts", bufs=1))
            ldp = ctx.enter_context(tc.tile_pool(name="ld", bufs=1))
            sqr = ctx.enter_context(tc.tile_pool(name="sqr", bufs=2))
            csp = ctx.enter_context(tc.tile_pool(name="csstage", bufs=4))
            mmp = ctx.enter_context(tc.tile_pool(name="mm", bufs=2, space="PSUM"))
            ep = ctx.enter_context(tc.tile_pool(name="etile", bufs=4))

            A = _Input(nc, ldp, xts, sqr, a_in, "a", AT_T,
                       nc.vector, nc.vector)
            P = _Input(nc, ldp, xts, sqr, p_in, "p", TFULL,
                       nc.gpsimd, nc.gpsimd)

            A.dma(0, CH)
            P.dma(0, CH)
            P.dma(CH, 2 * CH)
            P.dma(2 * CH, 3 * CH)
            P.dma(3 * CH, 4 * CH)
            A.dma(CH, 2 * CH)
            A.dma(2 * CH, AT_T)

            # Zero-padded K=128 stationaries: stat[par][:, m*128:(m+1)*128]
            # has the m-tile dims on K rows 64*par .. 64*par+64, 0 elsewhere.
            # Built with partition-moving SBUF->SBUF DMAs from xTs.
            stats = {}
            for inp, nm in ((A, "a"), (P, "p")):
                se = res.tile([128, MT * 128], F16, tag=f"stat_{nm}_e")
                so = res.tile([128, MT * 128], F16, tag=f"stat_{nm}_o")
                nc.vector.memset(se[:], 0.0)
                nc.vector.memset(so[:], 0.0)
                stats[nm] = (se, so)

            def emit_stats(inp, nm, eng):
                se, so = stats[nm]
                for m in range(MT):
                    src = inp.xTs[64 * (m % 2):64 * (m % 2) + 64,
                                  (m // 2) * 128:(m // 2 + 1) * 128]
                    eng.dma_start(
                        out=se[0:64, m * 128:(m + 1) * 128], in_=src)
                    eng.dma_start(
                        out=so[64:128, m * 128:(m + 1) * 128], in_=src)

            def emit_pair(si, pair):
                (matL, jL), (matR, jR) = pair
                fused = (matL == matR and jR == jL + 1)
                xi, yi = {"ap": (A, P), "aa": (A, A), "pp": (P, P)}[matL]
                se, so = stats["a" if xi is A else "p"]
                for m in range(MT):
                    mm_ps = mmp.tile([128, 2 * B], F32, tag="mm")
                    if fused:
                        # K=64 row-tiled: T0 (partitions 0:64, even tiles)
                        # and T8 (64:128, odd tiles) co-stream in the
                        # 64x128-tiled PE array.
                        for c in range(2):
                            for par, stt in ((0, se), (1, so)):
                                h0 = 64 * par
                                nc.tensor.matmul(
                                    mm_ps[:, par * B + c * 512:
                                          par * B + (c + 1) * 512],
                                    stt[h0:h0 + 64, m * 128:(m + 1) * 128],
                                    yi.xTs[h0:h0 + 64,
                                           (jL + c) * 512:(jL + c + 1) * 512],
                                    start=True, stop=True,
                                    tile_position=(h0, 0))
                    else:
                        # pair8: two independent 512-wide blocks
                        for h, (mat, j) in enumerate(pair):
                            xh, yh = {"ap": (A, P), "aa": (A, A),
                                      "pp": (P, P)}[mat]
                            seh, soh = stats["a" if xh is A else "p"]
                            for par, stt in ((0, seh), (1, soh)):
                                h0 = 64 * par
                                o0 = h * B + par * 512
                                nc.tensor.matmul(
                                    mm_ps[:, o0:o0 + 512],
                                    stt[h0:h0 + 64, m * 128:(m + 1) * 128],
                                    yh.xTs[h0:h0 + 64, j * 512:(j + 1) * 512],
                                    start=True, stop=True,
                                    tile_position=(h0, 0))
                    e = ep.tile([128, 2 * B], BF16, tag="e")
                    slot = ST_IDX(si, m)
                    nc.scalar.activation(
                        e[:], mm_ps[:], mybir.ActivationFunctionType.Exp,
                        scale=INV_T,
                        accum_out=st[:, slot:slot + 1],
                    )
                    if fused:
                        for h, (mat, j) in enumerate(pair):
                            ci = _COLACC.get((mat, j))
                            if ci is None:
                                continue
                            # block h: even half at h*512, odd at B+h*512
                            # (two contiguous [128,512] ops; strided 3D
                            # APs measured 30-100x slower on DVE)
                            for par in range(2):
                                eh = e[:, par * B + h * 512:
                                       par * B + (h + 1) * 512]
                                ca = colacc[:, ci, par * 512:(par + 1) * 512]
                                if m == 0:
                                    nc.vector.tensor_copy(ca, eh)
                                else:
                                    nc.vector.tensor_add(ca, ca, eh)
                # column sums leave as raw bf16 colacc tiles; the host
                # does the 128-partition reduce (frees PE/PSUM/DVE from
                # the reduction entirely).  gpsimd software-DGE queue so
                # the bulky SP transposes can't head-of-line block it.
                for h, (mat, j) in enumerate(pair):
                    ci = _COLACC.get((mat, j))
                    if ci is None:
                        continue
                    nc.gpsimd.dma_start(
                        out=o_cacc[:, ci * B:(ci + 1) * B],
                        in_=colacc[:, ci, :])

            # ---- woven schedule ----
            A.chunk(0, CH, nc.scalar)
            P.chunk(0, CH, nc.sync)
            emit_stats(A, "a", nc.gpsimd)
            emit_stats(P, "p", nc.sync)

            # diag cos(a_i,p_i) from raw f32 tiles + inverse norms
            dtmp = sqr.tile([128, CH, D], F32, tag="sq_ring")
            nc.vector.tensor_mul(dtmp[:, 0:MT, :], A.nat[:, 0:MT, :],
                                 P.nat[:, 0:MT, :])
            dots = sqr.tile([128, MT], F32, tag="ssq_ring")
            nc.vector.tensor_reduce(dots[:], dtmp[:, 0:MT, :],
                                    axis=mybir.AxisListType.X,
                                    op=mybir.AluOpType.add)
            nc.vector.tensor_mul(dots[:], dots[:], A.inv[:, 0:MT])
            nc.vector.tensor_mul(diag[:], dots[:], P.inv[:, 0:MT])

            # machine-matched self terms from the fp16 normalized tiles
            for inp, o_ssq in ((A, o_ssa), (P, o_ssp)):
                sqh = sqr.tile([128, MT, D], F32, tag="sq_ring")
                nc.vector.tensor_mul(sqh[:], inp.nat2h[:, 0:MT, :],
                                     inp.nat2h[:, 0:MT, :])
                ssq = sqr.tile([128, MT], F32, tag="ssq_ring")
                nc.vector.tensor_reduce(ssq[:], sqh[:],
                                        axis=mybir.AxisListType.X,
                                        op=mybir.AluOpType.add)
                nc.sync.dma_start(out=o_ssq[:], in_=ssq[:])

            # remaining chunks all emitted after pair0: their DVE/Pool
            # chain work lands early in those queues and the bulky SP
            # transposes spread across pairs 1-3, long before use
            P.chunk(CH, 2 * CH, nc.sync)
            emit_pair(0, PAIRS[0])
            P.chunk(2 * CH, 3 * CH, nc.sync)
            P.chunk(3 * CH, 4 * CH, nc.sync)
            emit_pair(1, PAIRS[1])
            A.chunk(CH, 2 * CH, nc.sync)
            A.chunk(2 * CH, AT_T, nc.sync)
            emit_pair(2, PAIRS[2])
            emit_pair(3, PAIRS[3])
            emit_pair(4, PAIRS[4])
            emit_pair(5, PAIRS[5])
            emit_pair(6, PAIRS[6])
            emit_pair(7, PAIRS[7])
            emit_pair(8, PAIRS[8])

            nc.sync.dma_start(out=o_st[:], in_=st[:])
            nc.sync.dma_start(out=o_diag[:], in_=diag[:])
    return nc


def combine(core_outs):
    """core_outs: list (per core) of dicts with the 9 output arrays."""
    rs = np.empty(N, np.float64)
    diag = np.empty(N, np.float32)
    self_terms = np.empty(N, np.float64)
    cs_ap_tot = np.zeros(N, np.float64)
    aa_contrib = np.zeros(N, np.float64)
    pp_contrib = np.zeros(N, np.float64)

    def unperm(vec_b):
        out = np.empty(B, np.float64)
        out[PERM_1024] = vec_b
        return out

    for k, o in enumerate(core_outs):
        sl = slice(k * B, (k + 1) * B)
        rs[sl] = o["st"].reshape(128, NSLOT, MT).astype(np.float64).sum(1) \
                        .T.reshape(B)
        diag[sl] = o["diag"].T.reshape(B)
        self_terms[sl] = (
            np.exp(INV_T * o["ssq_a"].astype(np.float64)) +
            np.exp(INV_T * o["ssq_p"].astype(np.float64))
        ).T.reshape(B)

        cacc = np.asarray(o["colacc"]).astype(np.float64) \
                 .reshape(128, NCS, B).sum(0)
        cs_ap = np.concatenate([unperm(cacc[j]) for j in range(8)])
        cs_ap_tot += np.roll(cs_ap, k * B)
        for row, j in enumerate(SYM_COL_BLOCKS):
            v = np.zeros(N, np.float64)
            v[j * B:(j + 1) * B] = unperm(cacc[8 + row])
            aa_contrib += np.roll(v, k * B)
            v = np.zeros(N, np.float64)
            v[j * B:(j + 1) * B] = unperm(cacc[11 + row])
            pp_contrib += np.roll(v, k * B)

    partition = (rs + cs_ap_tot + aa_contrib + pp_contrib - self_terms)
    pos_logit = INV_T * diag.astype(np.float64)
    loss = -(pos_logit - np.log(partition)).mean()
    return np.float32(loss)


def _split_waits(nc):
    """Walrus codegen allows ~1 sync wait per instruction; hoist extra
    waits onto same-engine NoOps inserted just before the instruction."""
    for fn in nc.m.functions:
        for blk in fn.blocks:
            new = []
            for inst in blk.instructions:
                si = getattr(inst, "sync_info", None)
                keep = 1
                if si is not None and si.on_wait and len(si.on_wait) > keep:
                    waits = list(si.on_wait)
                    for i, w in enumerate(waits[:-keep]):
                        nop = mybir.InstNoOp(name=f"{inst.name}-sw{i}")
                        nop.engine = inst.engine
                        nop.sync_info = mybir.SyncInfo(on_wait=[w], on_update=[])
                        new.append(nop)
                    inst.sync_info = mybir.SyncInfo(
                        on_wait=list(waits[-keep:]),
                        on_update=list(si.on_update))
                new.append(inst)
            blk.instructions = new


_NC_CACHE = None


def _get_program():
    global _NC_CACHE
    if _NC_CACHE is None:
        _NC_CACHE = build_program()
        _split_waits(_NC_CACHE)
    return _NC_CACHE


def run(anchor_embeddings, positive_embeddings, trace=False, **trace_kwargs):
    a = np.ascontiguousarray(anchor_embeddings, dtype=np.float32)
    p = np.ascontiguousarray(positive_embeddings, dtype=np.float32)
    in_maps = [
        {"a": np.roll(a, -k * B, axis=0), "p": np.roll(p, -k * B, axis=0)}
        for k in range(NCORES)
    ]
    nc = _get_program()
    res = run_bass_kernel_spmd(nc, in_maps, list(range(NCORES)), trace=trace,
                               **trace_kwargs)
    return combine(res.results), res


def kernel(anchor_embeddings, positive_embeddings):
    loss, _ = run(anchor_embeddings, positive_embeddings)
    return loss


# revision 28
# speedup vs baseline: 1.2547x; 1.0362x over previous
"""GTE contrastive loss kernel for 8 Trainium2 NeuronCores — v3.

Math (reference): loss = -mean_i( cos(a_i,p_i)/T - log(partition_i) ),
partition_i = sum_j E_ap[i,j] + sum_j E_aa[i,j] + sum_j E_ap[j,i]
            + sum_j E_pp[j,i] - (self_a)_i - (self_p)_i,
E_xy = exp(cos/T).  The self terms are exp(20*||x_hat_fp16||^2) computed
from the SAME fp16-rounded vectors the matmuls consume, so the device's
huge diagonal terms cancel exactly on the host at any matmul precision.

Sharding: core k owns row block k (1024 rows); inputs host-rotated by
-1024k rows (one SPMD program).  Column block j = global block (k+j)%8.
aa/pp symmetry: blocks 0..4; blocks 1..3 also emit column sums that
become the missing row-sum pieces on other cores.

v3 engine plan (driven by the v2 HW trace):
 - ALL transposes on the DMA engines: a 2-byte XBAR DMA-transpose of a
   [128, 128] fp16 slab (two adjacent 128-row tiles) yields the two
   transposed tiles STACKED in the partition dim (tile 2q on partitions
   0:64, tile 2q+1 on 64:128).  xTs keeps that stacked layout.
 - matmuls are fp16, K=128, with zero-padded stationaries (built by
   small partition-moving SBUF->SBUF DMAs): an "even" matmul carries the
   m-tile dims on K rows 0:64 (zeros below) so only the even tiles of
   the moving slab contribute; the "odd" one mirrors it.  A fused block
   pair (j, j+1) needs just 2 matmuls + 2 ldweights per [128, 2048]
   PSUM tile with 1024-wide moving operands.  Columns inside each block
   come out parity-permuted; the host unpermutes the column sums (row
   sums are order-invariant).
 - 9 block-pairs x 8 row-tiles, one [128,2048] Exp + f32 accum_out each.
 - exp output bf16; per-block column sums accumulate on the DVE (2-byte
   2x mode, strided slices); partition reduce on GPSIMD (axis=C).
 - startup pipelined in 16-tile chunks woven between the first pairs;
   a-tiles 40..64 are never used (aa stops at block 4) and are skipped.
"""

import os
import sys

import numpy as np

for _p in ("/opt/trn_rl_repo", os.path.expanduser("/root/.axon_site/_ro/trn_rl_repo")):
    if os.path.isdir(_p) and _p not in sys.path:
        sys.path.insert(0, _p)

from concourse import bass, tile  # noqa: E402
from concourse.bass_utils import run_bass_kernel_spmd  # noqa: E402

mybir = bass.mybir
F32 = mybir.dt.float32
F16 = mybir.dt.float16
BF16 = mybir.dt.bfloat16

N, D, NCORES = 8192, 64, 8
B = N // NCORES            # 1024 rows per core
MT = B // 128              # 8 row tiles of 128
TFULL = N // 128           # 64 tiles
INV_T = 20.0

AP_BLOCKS = list(range(8))
SYM_BLOCKS = [0, 1, 2, 3, 4]
SYM_COL_BLOCKS = [1, 2, 3]

PAIRS = [
    [("ap", 0), ("ap", 1)],
    [("ap", 2), ("ap", 3)],
    [("ap", 4), ("ap", 5)],
    [("ap", 6), ("ap", 7)],
    [("aa", 0), ("aa", 1)],
    [("aa", 2), ("aa", 3)],
    [("pp", 0), ("pp", 1)],
    [("pp", 2), ("pp", 3)],
    [("aa", 4), ("pp", 4)],
]
NSLOT = len(PAIRS)

_COLACC = {}
for _j in AP_BLOCKS:
    _COLACC[("ap", _j)] = _j
for _i, _j in enumerate(SYM_COL_BLOCKS):
    _COLACC[("aa", _j)] = 8 + _i
    _COLACC[("pp", _j)] = 11 + _i
NCS = len(_COLACC)

AT_T = 40                  # a tiles actually used (aT cols < 5120)
CH = 8                     # pipeline chunk, in 128-row tiles

# Column permutation inside one 1024-col block: raw slot s (as stored in
# colacc / cs outputs) holds block-local column PERM_1024[s].
# s = parity*512 + tau*128 + r  ->  col (2*tau + parity)*128 + r
PERM_1024 = np.empty(1024, np.int64)
for _s in range(1024):
    _par, _tau, _r = _s // 512, (_s % 512) // 128, _s % 128
    PERM_1024[_s] = (2 * _tau + _par) * 128 + _r


def ST_IDX(si, m):
    return si * MT + m


class _Input:
    """Chunked DMA -> normalize(fp16 out) -> stacked DMA-transpose."""

    def __init__(self, nc, ldp, xts, sqr, dram_in, name, tmax, sq_eng, mul_eng):
        self.nc, self.name, self.tmax = nc, name, tmax
        self.sq_eng, self.mul_eng = sq_eng, mul_eng
        self.sqr = sqr
        self.nat = ldp.tile([128, tmax, D], F32, tag=f"{name}_nat")
        self.nat2h = ldp.tile([128, tmax, D], F16, tag=f"{name}_nat2h")
        self.ss = ldp.tile([128, tmax], F32, tag=f"{name}_ss")
        self.nrm = ldp.tile([128, tmax], F32, tag=f"{name}_nrm")
        self.inv = ldp.tile([128, tmax], F32, tag=f"{name}_inv")
        self.src = dram_in[:].rearrange("(t p) d -> p t d", p=128)
        # stacked transposed layout: col q*128+r holds, on partitions
        # 0:64, dims of tile 2q row r; on 64:128, dims of tile 2q+1.
        self.xTs = xts.tile([128, (tmax // 2) * 128], F16, tag=f"{name}_xTs")

    def dma(self, c0, c1):
        self.nc.sync.dma_start(out=self.nat[:, c0:c1, :],
                               in_=self.src[:, c0:c1, :])

    def chunk(self, c0, c1, tr_eng):
        nc = self.nc
        w = c1 - c0
        sq = self.sqr.tile([128, CH, D], F32, tag="sq_ring")
        self.sq_eng.tensor_mul(sq[:, 0:w, :], self.nat[:, c0:c1, :],
                               self.nat[:, c0:c1, :])
        nc.vector.tensor_reduce(self.ss[:, c0:c1], sq[:, 0:w, :],
                                axis=mybir.AxisListType.X,
                                op=mybir.AluOpType.add)
        # rsqrt entirely on the DVE (an ACT Sqrt here head-of-line blocks
        # the exp stream in ACT's strict FIFO): bit-trick seed + 2 Newton
        # steps gives ~1e-6 relative — far below the fp16 rounding the
        # ssq self-term correction already absorbs.
        ssv = self.ss[:, c0:c1]
        yv = self.inv[:, c0:c1]
        yi = yv.bitcast(mybir.dt.int32)
        # seed: 0x5f3759df - (bits(x) >> 1)  ==  (bits(x)>>1)*-1 + C
        nc.vector.tensor_scalar(yi, ssv.bitcast(mybir.dt.int32),
                                1, None, mybir.AluOpType.arith_shift_right)
        nc.vector.tensor_scalar(yi, yi, -1, 0x5f3759df,
                                mybir.AluOpType.mult, mybir.AluOpType.add)
        t0 = self.sqr.tile([128, CH], F32, tag="nrs_ring")
        t0v = t0[:, 0:w]
        for _ in range(2):
            nc.vector.tensor_mul(t0v, yv, yv)            # y^2
            nc.vector.tensor_mul(t0v, t0v, ssv)          # x*y^2
            nc.vector.tensor_scalar(t0v, t0v, -0.5, 1.5,
                                    mybir.AluOpType.mult,
                                    mybir.AluOpType.add)  # 1.5-0.5xy^2
            nc.vector.tensor_mul(yv, yv, t0v)            # y *= corr
        inv_b = yv.broadcast_to([128, w, D])
        self.mul_eng.tensor_mul(self.nat2h[:, c0:c1, :],
                                self.nat[:, c0:c1, :], inv_b)
        for q0 in range(c0 // 2, c1 // 2):
            tr_eng.dma_start(
                out=self.xTs[:, q0 * 128:(q0 + 1) * 128],
                in_=self.nat2h[:, 2 * q0:2 * q0 + 2, :],
                transpose=True,
            )


def build_program():
    nc = bass.Bass()
    a_in = nc.declare_dram_parameter("a", [N, D], F32, isOutput=False)
    p_in = nc.declare_dram_parameter("p", [N, D], F32, isOutput=False)
    o_st = nc.declare_dram_parameter("st", [128, MT * NSLOT], F32, isOutput=True)
    o_diag = nc.declare_dram_parameter("diag", [128, MT], F32, isOutput=True)
    o_ssa = nc.declare_dram_parameter("ssq_a", [128, MT], F32, isOutput=True)
    o_ssp = nc.declare_dram_parameter("ssq_p", [128, MT], F32, isOutput=True)
    o_cacc = nc.declare_dram_parameter("colacc", [128, NCS * B], BF16,
                                       isOutput=True)

    with tile.TileContext(nc) as tc:
        import contextlib

        with contextlib.ExitStack() as ctx:
            res = ctx.enter_context(tc.tile_pool(name="results", bufs=1))
            st = res.tile([128, MT * NSLOT], F32)
            diag = res.tile([128, MT], F32)
            colacc = res.tile([128, NCS, B], BF16)

            xts = ctx.enter_context(tc.tile_pool(name="xts", bufs=1))
            ldp = ctx.enter_context(tc.tile_pool(name="ld", bufs=1))
            sqr = ctx.enter_context(tc.tile_pool(name="sqr", bufs=2))
            mmp = ctx.enter_context(tc.tile_pool(name="mm", bufs=2, space="PSUM"))
            ep = ctx.enter_context(tc.tile_pool(name="etile", bufs=4))

            A = _Input(nc, ldp, xts, sqr, a_in, "a", AT_T,
                       nc.vector, nc.vector)
            P = _Input(nc, ldp, xts, sqr, p_in, "p", TFULL,
                       nc.gpsimd, nc.gpsimd)

            A.dma(0, CH)
            P.dma(0, CH)
            P.dma(CH, 4 * CH)
            P.dma(4 * CH, 8 * CH)
            A.dma(CH, AT_T)

            # Zero-padded K=128 stationaries: stat[par][:, m*128:(m+1)*128]
            # has the m-tile dims on K rows 64*par .. 64*par+64, 0 elsewhere.
            # Built with partition-moving SBUF->SBUF DMAs from xTs.
            stats = {}
            for inp, nm in ((A, "a"), (P, "p")):
                se = res.tile([128, MT * 128], F16, tag=f"stat_{nm}_e")
                so = res.tile([128, MT * 128], F16, tag=f"stat_{nm}_o")
                nc.vector.memset(se[:], 0.0)
                nc.vector.memset(so[:], 0.0)
                stats[nm] = (se, so)

            def emit_stats(inp, nm, eng):
                se, so = stats[nm]
                for m in range(MT):
                    src = inp.xTs[64 * (m % 2):64 * (m % 2) + 64,
                                  (m // 2) * 128:(m // 2 + 1) * 128]
                    eng.dma_start(
                        out=se[0:64, m * 128:(m + 1) * 128], in_=src)
                    eng.dma_start(
                        out=so[64:128, m * 128:(m + 1) * 128], in_=src)

            def emit_pair(si, pair):
                (matL, jL), (matR, jR) = pair
                fused = (matL == matR and jR == jL + 1)
                xi, yi = {"ap": (A, P), "aa": (A, A), "pp": (P, P)}[matL]
                se, so = stats["a" if xi is A else "p"]
                for m in range(MT):
                    mm_ps = mmp.tile([128, 2 * B], F32, tag="mm")
                    if fused:
                        # K=64 row-tiled: T0 (partitions 0:64, even tiles)
                        # and T8 (64:128, odd tiles) co-stream in the
                        # 64x128-tiled PE array.
                        for c in range(2):
                            for par, stt in ((0, se), (1, so)):
                                h0 = 64 * par
                                nc.tensor.matmul(
                                    mm_ps[:, par * B + c * 512:
                                          par * B + (c + 1) * 512],
                                    stt[h0:h0 + 64, m * 128:(m + 1) * 128],
                                    yi.xTs[h0:h0 + 64,
                                           (jL + c) * 512:(jL + c + 1) * 512],
                                    start=True, stop=True,
                                    tile_position=(h0, 0))
                    else:
                        # pair8: two independent 512-wide blocks
                        for h, (mat, j) in enumerate(pair):
                            xh, yh = {"ap": (A, P), "aa": (A, A),
                                      "pp": (P, P)}[mat]
                            seh, soh = stats["a" if xh is A else "p"]
                            for par, stt in ((0, seh), (1, soh)):
                                h0 = 64 * par
                                o0 = h * B + par * 512
                                nc.tensor.matmul(
                                    mm_ps[:, o0:o0 + 512],
                                    stt[h0:h0 + 64, m * 128:(m + 1) * 128],
                                    yh.xTs[h0:h0 + 64, j * 512:(j + 1) * 512],
                                    start=True, stop=True,
                                    tile_position=(h0, 0))
                    e = ep.tile([128, 2 * B], BF16, tag="e")
                    slot = ST_IDX(si, m)
                    nc.scalar.activation(
                        e[:], mm_ps[:], mybir.ActivationFunctionType.Exp,
                        scale=INV_T,
                        accum_out=st[:, slot:slot + 1],
                    )
                    if fused:
                        for h, (mat, j) in enumerate(pair):
                            ci = _COLACC.get((mat, j))
                            if ci is None:
                                continue
                            # block h: even half at h*512, odd at B+h*512
                            # (two contiguous [128,512] ops; strided 3D
                            # APs measured 30-100x slower on DVE)
                            for par in range(2):
                                eh = e[:, par * B + h * 512:
                                       par * B + (h + 1) * 512]
                                ca = colacc[:, ci, par * 512:(par + 1) * 512]
                                if m == 0:
                                    nc.vector.tensor_copy(ca, eh)
                                else:
                                    nc.vector.tensor_add(ca, ca, eh)
                # column sums leave as raw bf16 colacc tiles; the host
                # does the 128-partition reduce (frees PE/PSUM/DVE from
                # the reduction entirely).  gpsimd software-DGE queue so
                # the bulky SP transposes can't head-of-line block it.
                for h, (mat, j) in enumerate(pair):
                    ci = _COLACC.get((mat, j))
                    if ci is None:
                        continue
                    nc.gpsimd.dma_start(
                        out=o_cacc[:, ci * B:(ci + 1) * B],
                        in_=colacc[:, ci, :])

            # ---- woven schedule ----
            A.chunk(0, CH, nc.scalar)
            P.chunk(0, CH, nc.sync)
            emit_stats(A, "a", nc.gpsimd)
            emit_stats(P, "p", nc.sync)

            # diag cos(a_i,p_i) from raw f32 tiles + inverse norms
            dtmp = sqr.tile([128, CH, D], F32, tag="sq_ring")
            nc.vector.tensor_mul(dtmp[:, 0:MT, :], A.nat[:, 0:MT, :],
                                 P.nat[:, 0:MT, :])
            dots = sqr.tile([128, MT], F32, tag="ssq_ring")
            nc.vector.tensor_reduce(dots[:], dtmp[:, 0:MT, :],
                                    axis=mybir.AxisListType.X,
                                    op=mybir.AluOpType.add)
            nc.vector.tensor_mul(dots[:], dots[:], A.inv[:, 0:MT])
            nc.vector.tensor_mul(diag[:], dots[:], P.inv[:, 0:MT])

            # machine-matched self terms from the fp16 normalized tiles
            for inp, o_ssq in ((A, o_ssa), (P, o_ssp)):
                sqh = sqr.tile([128, MT, D], F32, tag="sq_ring")
                nc.vector.tensor_mul(sqh[:], inp.nat2h[:, 0:MT, :],
                                     inp.nat2h[:, 0:MT, :])
                ssq = sqr.tile([128, MT], F32, tag="ssq_ring")
                nc.vector.tensor_reduce(ssq[:], sqh[:],
                                        axis=mybir.AxisListType.X,
                                        op=mybir.AluOpType.add)
                nc.sync.dma_start(out=o_ssq[:], in_=ssq[:])

            # 8-tile chunks woven so each chunk's transpose DMAs finish
            # during the pair BEFORE its consumer: the DMA-completion
            # semaphore is a batched per-queue counter, so every pair
            # waits on all previously emitted transposes — they must
            # complete early, but not be emitted too early.
            P.chunk(CH, 2 * CH, nc.sync)
            emit_pair(0, PAIRS[0])
            P.chunk(2 * CH, 3 * CH, nc.sync)
            P.chunk(3 * CH, 4 * CH, nc.sync)
            emit_pair(1, PAIRS[1])
            P.chunk(4 * CH, 5 * CH, nc.sync)
            P.chunk(5 * CH, 6 * CH, nc.sync)
            emit_pair(2, PAIRS[2])
            P.chunk(6 * CH, 7 * CH, nc.sync)
            P.chunk(7 * CH, 8 * CH, nc.sync)
            emit_pair(3, PAIRS[3])
            A.chunk(CH, 2 * CH, nc.sync)
            emit_pair(4, PAIRS[4])
            A.chunk(2 * CH, 3 * CH, nc.sync)
            A.chunk(3 * CH, 4 * CH, nc.sync)
            emit_pair(5, PAIRS[5])
            A.chunk(4 * CH, 5 * CH, nc.sync)
            emit_pair(6, PAIRS[6])
            emit_pair(7, PAIRS[7])
            emit_pair(8, PAIRS[8])

            nc.sync.dma_start(out=o_st[:], in_=st[:])
            nc.sync.dma_start(out=o_diag[:], in_=diag[:])
    return nc


def combine(core_outs):
    """core_outs: list (per core) of dicts with the 9 output arrays."""
    rs = np.empty(N, np.float64)
    diag = np.empty(N, np.float32)
    self_terms = np.empty(N, np.float64)
    cs_ap_tot = np.zeros(N, np.float64)
    aa_contrib = np.zeros(N, np.float64)
    pp_contrib = np.zeros(N, np.float64)

    def unperm(vec_b):
        out = np.empty(B, np.float64)
        out[PERM_1024] = vec_b
        return out

    for k, o in enumerate(core_outs):
        sl = slice(k * B, (k + 1) * B)
        rs[sl] = o["st"].reshape(128, NSLOT, MT).astype(np.float64).sum(1) \
                        .T.reshape(B)
        diag[sl] = o["diag"].T.reshape(B)
        self_terms[sl] = (
            np.exp(INV_T * o["ssq_a"].astype(np.float64)) +
            np.exp(INV_T * o["ssq_p"].astype(np.float64))
        ).T.reshape(B)

        cacc = np.asarray(o["colacc"]).astype(np.float64) \
                 .reshape(128, NCS, B).sum(0)
        cs_ap = np.concatenate([unperm(cacc[j]) for j in range(8)])
        cs_ap_tot += np.roll(cs_ap, k * B)
        for row, j in enumerate(SYM_COL_BLOCKS):
            v = np.zeros(N, np.float64)
            v[j * B:(j + 1) * B] = unperm(cacc[8 + row])
            aa_contrib += np.roll(v, k * B)
            v = np.zeros(N, np.float64)
            v[j * B:(j + 1) * B] = unperm(cacc[11 + row])
            pp_contrib += np.roll(v, k * B)

    partition = (rs + cs_ap_tot + aa_contrib + pp_contrib - self_terms)
    pos_logit = INV_T * diag.astype(np.float64)
    loss = -(pos_logit - np.log(partition)).mean()
    return np.float32(loss)


def _split_waits(nc):
    """Walrus codegen allows ~1 sync wait per instruction; hoist extra
    waits onto same-engine NoOps inserted just before the instruction."""
    for fn in nc.m.functions:
        for blk in fn.blocks:
            new = []
            for inst in blk.instructions:
                si = getattr(inst, "sync_info", None)
                keep = 1
                if si is not None and si.on_wait and len(si.on_wait) > keep:
                    waits = list(si.on_wait)
                    for i, w in enumerate(waits[:-keep]):
                        nop = mybir.InstNoOp(name=f"{inst.name}-sw{i}")
                        nop.engine = inst.engine
                        nop.sync_info = mybir.SyncInfo(on_wait=[w], on_update=[])
                        new.append(nop)
                    inst.sync_info = mybir.SyncInfo(
                        on_wait=list(waits[-keep:]),
                        on_update=list(si.on_update))
                new.append(inst)
            blk.instructions = new


_NC_CACHE = None


def _get_program():
    global _NC_CACHE
    if _NC_CACHE is None:
        _NC_CACHE = build_program()
        _split_waits(_NC_CACHE)
    return _NC_CACHE


def run(anchor_embeddings, positive_embeddings, trace=False, **trace_kwargs):
    a = np.ascontiguousarray(anchor_embeddings, dtype=np.float32)
    p = np.ascontiguousarray(positive_embeddings, dtype=np.float32)
    in_maps = [
        {"a": np.roll(a, -k * B, axis=0), "p": np.roll(p, -k * B, axis=0)}
        for k in range(NCORES)
    ]
    nc = _get_program()
    res = run_bass_kernel_spmd(nc, in_maps, list(range(NCORES)), trace=trace,
                               **trace_kwargs)
    return combine(res.results), res


def kernel(anchor_embeddings, positive_embeddings):
    loss, _ = run(anchor_embeddings, positive_embeddings)
    return loss
